# revision 1
# baseline (speedup 1.0000x reference)
"""Trainium2 Bass kernel for AdaptiveDiffAttention.

Pure data parallel across 8 NeuronCores: each core processes B/8 = 2048
samples with a replicated copy of the weights (cast to bf16 on host for
TensorEngine throughput). No collectives.

Math per sample b (seq len 2, heads 4, head dim 256):
  tokens  = x.reshape(2, 1024)
  lam     = sigmoid(relu(x @ L1) @ L2)
  Q,K,V   = tokens @ W*  (per token)
  softmax over 2 keys => a_q = sigmoid(SCALE * <Q_q, K_0 - K_1>)
  w0_q = relu(a1_q - lam*a2_q); w1_q = relu((1-a1_q) - lam*(1-a2_q))
  A_q  = w0_q * V_0 + w1_q * V_1   (per head)
  out_q = A_q @ WO + tokens_q

Host-side prep inside kernel(): shard x, cast weights to bf16, and build
a transposed bf16 copy of x (feature-major) so the kernel needs no
on-chip transposes; the attention combine runs on the PE as
V.T @ diag(w) matmuls, producing A^T directly for the WO matmul.
"""

import sys

for _p in ("/opt/trn_rl_repo", "/root/.axon_site/_ro/trn_rl_repo"):
    if _p not in sys.path:
        sys.path.append(_p)

import numpy as np
import ml_dtypes

import concourse.bass as bass
import concourse.mybir as mybir
import bass_rust
from concourse.tile import TileContext
from concourse.masks import make_identity

F32 = mybir.dt.float32
BF16 = mybir.dt.bfloat16

DIM = 2048
SD = 1024
H = 4
HD = 256
LH = 256
SCALE = HD ** -0.5
N_CORES = 8
B_FULL = 16384
B_CORE = B_FULL // N_CORES  # 2048

AluOp = mybir.AluOpType
ActFn = mybir.ActivationFunctionType


def split_excess_waits(nc, max_waits=1):
    """Walrus codegen in this container rejects >1 sync wait on CTRL-class
    instructions. Move excess waits onto chained nops before the offender."""
    for f in nc.m.functions:
        for bb in f.blocks:
            new_insts = []
            for inst in bb.instructions:
                si = inst.sync_info
                if si is not None and si.on_wait and len(si.on_wait) > max_waits:
                    waits = list(si.on_wait)
                    extra, keep = waits[:-max_waits], waits[-max_waits:]
                    for ci in range(0, len(extra), max_waits):
                        chunk = extra[ci:ci + max_waits]
                        nop = mybir.InstNoOp(name=f"{inst.name}-wsplit{ci}")
                        nop.engine = inst.engine
                        nop.sync_info = bass_rust.SyncInfo(
                            on_wait=chunk, on_update=[])
                        nc.register_instruction(nop, overwrite=True)
                        new_insts.append(nop)
                    inst.sync_info = bass_rust.SyncInfo(
                        on_wait=keep, on_update=list(si.on_update or []))
                new_insts.append(inst)
            bb.instructions = new_insts


def build_kernel(n_samples=B_CORE, repeats=1):
    """Build the single-core Bass graph. n_samples must be a multiple of 128."""
    assert n_samples % 128 == 0
    n_mtiles = n_samples // 128

    nc = bass.Bass()

    x_d = nc.declare_dram_parameter("x", [n_samples, DIM], F32, isOutput=False)
    # x transposed per m-tile: [mt, feat_in_tile(p), ftile, b] bf16
    xtp_d = nc.declare_dram_parameter(
        "xtp", [n_mtiles, 128, 16, 128], BF16, isOutput=False)
    w_d = {}
    for name, pname in (("q1", "WQ1_w"), ("k1", "WK1_w"), ("q2", "WQ2_w"),
                        ("k2", "WK2_w"), ("v", "WV_w"), ("o", "WO_w")):
        w_d[name] = nc.declare_dram_parameter(pname, [SD, SD], BF16,
                                              isOutput=False)
    l1_d = nc.declare_dram_parameter("L1_w", [DIM, LH], BF16, isOutput=False)
    l2r_d = nc.declare_dram_parameter("L2r", [128, LH], F32, isOutput=False)
    out_d = nc.declare_dram_parameter("out", [n_samples, DIM], F32, isOutput=True)

    with TileContext(nc) as tc:
        with (
            tc.tile_pool(name="const", bufs=1) as const,
            tc.tile_pool(name="xnat", bufs=2) as xnat_p,
            tc.tile_pool(name="xt", bufs=2) as xt_p,
            tc.tile_pool(name="qk", bufs=1) as qk_p,
            tc.tile_pool(name="kdp", bufs=2) as kd_p,
            tc.tile_pool(name="vbuf", bufs=2) as v_p,
            tc.tile_pool(name="ptmp", bufs=2) as ptmp_p,
            tc.tile_pool(name="small", bufs=2) as small_p,
            tc.tile_pool(name="hbuf", bufs=2) as h_p,
            tc.tile_pool(name="dpool", bufs=20) as d_p,
            tc.tile_pool(name="at", bufs=2) as at_p,
            tc.tile_pool(name="obuf", bufs=2) as o_p,
            tc.tile_pool(name="psum_big", bufs=3, space="PSUM") as ps_big,
            tc.tile_pool(name="psum_lam", bufs=2, space="PSUM") as ps_lam_p,
        ):
            # ---------------- resident weights (already bf16) ----------------
            w_sb = {}
            for name in ("q1", "k1", "q2", "k2", "v", "o"):
                wt = const.tile([128, 8, SD], BF16, name=f"w_{name}")
                wr = w_d[name].rearrange("(ko p) n -> p ko n", p=128)
                # split so early-k matmuls can start before the full DMA lands
                nc.sync.dma_start(wt[:, :4, :], wr[:, :4, :])
                nc.sync.dma_start(wt[:, 4:, :], wr[:, 4:, :])
                w_sb[name] = wt
            l1_sb = const.tile([128, 16, LH], BF16, name="l1")
            nc.sync.dma_start(
                l1_sb[:], l1_d.rearrange("(ko p) n -> p ko n", p=128))
            l2_rep = const.tile([128, LH], F32, name="l2rep")
            nc.sync.dma_start(l2_rep[:], l2r_d[:])

            id_bf16 = const.tile([128, 128], BF16, name="id16")
            make_identity(nc, id_bf16[:])

            # round-robin plain psum->sbuf eviction between ACT and DVE
            evict_ctr = [0]

            def evict(dst, src):
                evict_ctr[0] += 1
                if evict_ctr[0] % 3 == 0:
                    nc.vector.tensor_copy(dst, src)
                else:
                    nc.scalar.copy(dst, src)

            # ---------------- main loop over 128-sample tiles ----------------
            for mt_rep in range(n_mtiles * repeats):
                mt = mt_rep % n_mtiles
                r0 = mt * 128

                x_nat = xnat_p.tile([128, DIM], F32, tag="xnat", name="xnat")
                nc.sync.dma_start(x_nat[:], x_d[r0:r0 + 128, :])
                xt = xt_p.tile([128, 16, 128], BF16, tag="xt", name="xt")
                nc.sync.dma_start(xt[:], xtp_d[mt])

                # Projections. Q/K/V per token in natural layout [b, f].
                # lam MLP hidden accumulates across both tokens (full x row).
                ps_lam = ps_lam_p.tile([128, LH], F32, tag="plam", name="plam")
                proj = {}  # (w, tok) -> sbuf tile [128, 1024] bf16
                for tok in range(2):
                    for name in ("q1", "q2", "v"):
                        pool = v_p if name == "v" else qk_p
                        proj[(name, tok)] = pool.tile(
                            [128, SD], BF16, tag=f"qkv_{name}_{tok}",
                            name=f"qkv_{name}_{tok}")
                # token difference of x^T: Kdiff_s = (T0 - T1) @ WKs directly
                xtd = ptmp_p.tile([128, 8, 128], BF16, tag="xtd", name="xtd")
                nc.vector.tensor_tensor(xtd[:], xt[:, 0:8, :], xt[:, 8:16, :],
                                        AluOp.subtract)
                kdiff = {}
                for kname in ("k1", "k2"):
                    psw = ps_big.tile([128, SD], F32, tag="pbig", name="pbig")
                    for fi in range(8):
                        for n in range(2):
                            nsl = slice(n * 512, (n + 1) * 512)
                            nc.tensor.matmul(
                                psw[:, nsl], xtd[:, fi, :],
                                w_sb[kname][:, fi, nsl],
                                start=(fi == 0), stop=(fi == 7))
                    kd = kd_p.tile([128, SD], BF16, tag=f"kd_{kname}",
                                   name=f"kd_{kname}")
                    kdiff[kname] = kd
                    evict(kd[:], psw[:])
                # lam MLP early so sigmoid/weight chain overlaps Q/V GEMMs
                for tok in range(2):
                    for fi in range(8):
                        nc.tensor.matmul(
                            ps_lam[:], xt[:, tok * 8 + fi, :],
                            l1_sb[:, tok * 8 + fi, :],
                            start=(tok == 0 and fi == 0),
                            stop=(tok == 1 and fi == 7))
                for tok in range(2):
                    for name in ("q1", "q2", "v"):
                        psw = ps_big.tile([128, SD], F32, tag="pbig",
                                          name="pbig")
                        for fi in range(8):
                            lhsT = xt[:, tok * 8 + fi, :]
                            for n in range(2):
                                nsl = slice(n * 512, (n + 1) * 512)
                                nc.tensor.matmul(
                                    psw[:, nsl], lhsT,
                                    w_sb[name][:, fi, nsl],
                                    start=(fi == 0), stop=(fi == 7))
                        evict(proj[(name, tok)][:], psw[:])

                # lambda = sigmoid(relu(H) . L2)
                h_sb = h_p.tile([128, LH], F32, tag="h", name="h")
                nc.scalar.activation(h_sb[:], ps_lam[:], ActFn.Relu)
                nc.vector.tensor_tensor(h_sb[:], h_sb[:], l2_rep[:], AluOp.mult)
                logit = small_p.tile([128, 1], F32, tag="logit", name="logit")
                nc.vector.tensor_reduce(logit[:], h_sb[:],
                                        axis=mybir.AxisListType.X, op=AluOp.add)
                lam = small_p.tile([128, 1], F32, tag="lam", name="lam")
                nc.scalar.activation(lam[:], logit[:], ActFn.Sigmoid)
                u = small_p.tile([128, 1], F32, tag="u", name="u")
                nc.vector.tensor_scalar(u[:], lam[:], -1.0, 1.0,
                                        AluOp.mult, AluOp.add)

                # scores: r = <Q_q, Kdiff> per head; a = sigmoid(SCALE*r)
                a = {}
                for si, sname in enumerate(("1", "2")):
                    for q in range(2):
                        p = ptmp_p.tile([128, SD], BF16, tag="p", name="p")
                        nc.vector.tensor_tensor(
                            p[:], proj[(f"q{sname}", q)][:],
                            kdiff[f"k{sname}"][:], AluOp.mult)
                        r = small_p.tile([128, H], F32, tag=f"r{si}{q}",
                                         name=f"r{si}{q}")
                        nc.vector.tensor_reduce(
                            r[:], p.rearrange("b (h d) -> b h d", h=H),
                            axis=mybir.AxisListType.X, op=AluOp.add)
                        aa = small_p.tile([128, H], F32, tag=f"a{si}{q}",
                                          name=f"a{si}{q}")
                        nc.scalar.activation(aa[:], r[:], ActFn.Sigmoid,
                                             scale=float(SCALE))
                        a[(si, q)] = aa

                # diff-attn weights -> diag matrices (bf16)
                dmats = {}
                for q in range(2):
                    t = small_p.tile([128, H], F32, tag=f"t{q}", name=f"t{q}")
                    nc.vector.tensor_scalar_mul(t[:], a[(1, q)][:], lam[:])
                    w0q = small_p.tile([128, H], F32, tag=f"w0{q}",
                                       name=f"w0{q}")
                    nc.vector.tensor_tensor(w0q[:], a[(0, q)][:], t[:],
                                            AluOp.subtract)
                    nc.vector.tensor_scalar_max(w0q[:], w0q[:], 0.0)
                    w1q = small_p.tile([128, H], F32, tag=f"w1{q}",
                                       name=f"w1{q}")
                    nc.vector.tensor_tensor(w1q[:], t[:], a[(0, q)][:],
                                            AluOp.subtract)
                    nc.vector.tensor_scalar(w1q[:], w1q[:], u[:], 0.0,
                                            AluOp.add, AluOp.max)
                    for h in range(H):
                        d0 = d_p.tile([128, 128], BF16, tag="dmat", name="dmat")
                        nc.vector.tensor_scalar_mul(
                            d0[:], id_bf16[:], w0q[:, h:h + 1])
                        d1 = d_p.tile([128, 128], BF16, tag="dmat", name="dmat")
                        nc.vector.tensor_scalar_mul(
                            d1[:], id_bf16[:], w1q[:, h:h + 1])
                        dmats[(q, h, 0)] = d0
                        dmats[(q, h, 1)] = d1

                # A_q^T via diag matmuls, 4 tiles packed per psum bank:
                #   A_q^T[ftile] = V_0[:,ft].T @ D0[q,h] + V_1[:,ft].T @ D1[q,h]
                # One stationary V tile serves both q's psum banks.
                at = {}
                for q in range(2):
                    at[q] = at_p.tile([128, 8, 128], BF16, tag=f"at{q}",
                                      name=f"at{q}")
                    psq = ps_big.tile([128, SD], F32, tag="pbig", name="pbig")
                    for ft in range(8):
                        h = ft // 2
                        fsl = slice(ft * 128, (ft + 1) * 128)
                        for kv in range(2):
                            nc.tensor.matmul(
                                psq[:, fsl], proj[("v", kv)][:, fsl],
                                dmats[(q, h, kv)][:],
                                start=(kv == 0), stop=(kv == 1),
                                skip_group_check=(ft % 4 != 0))
                    evict(at[q][:],
                          psq[:].rearrange("b (f c) -> b f c", c=128))

                # out_q = A_q @ WO + tokens_q
                for q in range(2):
                    o_sb = o_p.tile([128, SD], F32, tag=f"o{q}", name=f"o{q}")
                    pso = ps_big.tile([128, SD], F32, tag="pbig", name="pbig")
                    for fi in range(8):
                        for n in range(2):
                            nsl = slice(n * 512, (n + 1) * 512)
                            nc.tensor.matmul(pso[:, nsl], at[q][:, fi, :],
                                             w_sb["o"][:, fi, nsl],
                                             start=(fi == 0), stop=(fi == 7))
                    osl = slice(q * SD, (q + 1) * SD)
                    nc.vector.tensor_tensor(o_sb[:], pso[:],
                                            x_nat[:, osl], AluOp.add)
                    nc.sync.dma_start(out_d[r0:r0 + 128, osl], o_sb[:])

    split_excess_waits(nc)
    return nc


_NC_CACHE = {}


def _get_nc(n_samples):
    if n_samples not in _NC_CACHE:
        _NC_CACHE[n_samples] = build_kernel(n_samples)
    return _NC_CACHE[n_samples]


def host_prep(inputs, n_samples=B_CORE):
    """Host-side shard + dtype/layout prep. Returns in_maps for 8 cores."""
    x = np.ascontiguousarray(np.asarray(inputs["x"], dtype=np.float32))
    assert x.shape[0] == N_CORES * n_samples and x.shape[1] == DIM
    bf = ml_dtypes.bfloat16
    ws = {}
    for k in ("WQ1_w", "WK1_w", "WQ2_w", "WK2_w", "WV_w", "WO_w", "L1_w"):
        ws[k] = np.ascontiguousarray(np.asarray(inputs[k]).astype(bf))
    l2rep = np.ascontiguousarray(
        np.broadcast_to(np.asarray(inputs["L2_w"], dtype=np.float32)
                        .reshape(1, LH), (128, LH)))
    n_mtiles = n_samples // 128
    # cast once (contiguous, fast), then one big strided transpose in bf16:
    # [core, mt, b, ft, p] -> [core, mt, p, ft, b]
    xb = x.astype(bf).reshape(N_CORES, n_mtiles, 128, 16, 128)
    xtp_all = np.ascontiguousarray(xb.transpose(0, 1, 4, 3, 2))
    in_maps = []
    for c in range(N_CORES):
        m = {"x": np.ascontiguousarray(x[c * n_samples:(c + 1) * n_samples]),
             "xtp": xtp_all[c], "L2r": l2rep}
        m.update(ws)
        in_maps.append(m)
    return in_maps


def kernel(**inputs):
    from concourse.bass_utils import run_bass_kernel_spmd

    nc = _get_nc(B_CORE)
    in_maps = host_prep(inputs, B_CORE)
    res = run_bass_kernel_spmd(nc, in_maps, core_ids=list(range(N_CORES)))
    return np.concatenate([res.results[c]["out"] for c in range(N_CORES)], axis=0)



# revision 18
# speedup vs baseline: 5.6463x; 5.6463x over previous
"""Trainium2 Bass kernel for AdaptiveDiffAttention.

Pure data parallel across 8 NeuronCores: each core processes B/8 = 2048
samples with a replicated copy of the weights. No collectives.

Math per sample b (seq len 2, heads 4, head dim 256):
  tokens  = x.reshape(2, 1024)
  lam     = sigmoid(relu(x @ L1) @ L2)
  Q,K,V   = tokens @ W*  (per token)
  softmax over 2 keys => a_q = sigmoid(SCALE * <Q_q, K_0 - K_1>)
  w0_q = relu(a1_q - lam*a2_q); w1_q = relu((1-a1_q) - lam*(1-a2_q))
  A_q  = w0_q * V_0 + w1_q * V_1   (per head)
  out_q = A_q @ WO + tokens_q

All big GEMMs run as fp8e4 DoubleRow matmuls (2 K-chunks per MM, 2x
stream rate). Scale management (TRN e4m3 normal range [2^-6, 240]):
  x8 = 16*x, W' = 64*W  =>  Q'/Kd' psum = 1024*(Q/Kd)   (kept in bf16)
  score sigmoid folds SCALE/2^20; V evicted with 1/1024 to natural bf16;
  A evicted with x8 to fp8 (stationary for the fp8 WO matmul);
  WO' = 64*WO => psum = 512*out_attn, final ACT copy folds 1/512.
The lam MLP's L1 GEMM is folded into the Q1 GEMM loops (same stationary
x^T chunk feeds a third moving operand), so it costs no extra weight
loads; L2' = L2/1024 on host.

Host-side prep inside kernel(): shard x, scale+cast weights to fp8, and
build a transposed fp8 copy of x (feature-major) so the kernel needs no
on-chip transposes; the attention combine runs on the PE as
V.T @ diag(w) matmuls, producing A^T directly for the WO matmul.
"""

import contextlib
import sys

for _p in ("/opt/trn_rl_repo", "/root/.axon_site/_ro/trn_rl_repo"):
    if _p not in sys.path:
        sys.path.append(_p)

import numpy as np
import ml_dtypes

import concourse.bass as bass
import concourse.mybir as mybir
import bass_rust
from concourse.tile import TileContext
from concourse.masks import make_identity

F32 = mybir.dt.float32
BF16 = mybir.dt.bfloat16
FP8 = mybir.dt.float8e4

DIM = 2048
SD = 1024
H = 4
HD = 256
LH = 256
SCALE = HD ** -0.5
N_CORES = 8
B_FULL = 16384
B_CORE = B_FULL // N_CORES  # 2048

X_SCALE = 16.0
W_SCALE = 64.0
QK_SCALE = X_SCALE * W_SCALE          # 1024: Q'/Kd'/V' = 1024 * natural
A_SCALE = 8.0                          # at' = 8*A in fp8
O_SCALE = A_SCALE * W_SCALE            # 512: WO psum = 512 * out_attn

AluOp = mybir.AluOpType
ActFn = mybir.ActivationFunctionType
DR = mybir.MatmulPerfMode.DoubleRow


def split_excess_waits(nc, max_waits=1):
    """Walrus codegen in this container rejects >1 sync wait on CTRL-class
    instructions. Move excess waits onto chained nops before the offender."""
    for f in nc.m.functions:
        for bb in f.blocks:
            new_insts = []
            for inst in bb.instructions:
                si = inst.sync_info
                if si is not None and si.on_wait and len(si.on_wait) > max_waits:
                    waits = list(si.on_wait)
                    extra, keep = waits[:-max_waits], waits[-max_waits:]
                    for ci in range(0, len(extra), max_waits):
                        chunk = extra[ci:ci + max_waits]
                        nop = mybir.InstNoOp(name=f"{inst.name}-wsplit{ci}")
                        nop.engine = inst.engine
                        nop.sync_info = bass_rust.SyncInfo(
                            on_wait=chunk, on_update=[])
                        nc.register_instruction(nop, overwrite=True)
                        new_insts.append(nop)
                    inst.sync_info = bass_rust.SyncInfo(
                        on_wait=keep, on_update=list(si.on_update or []))
                new_insts.append(inst)
            bb.instructions = new_insts


def build_kernel(n_samples=B_CORE, repeats=1, hw_repeats=1):
    """Build the single-core Bass graph. n_samples must be a multiple of 128.

    repeats: python-unrolled extra passes over the same tiles (graph grows).
    hw_repeats: hardware For_i loop around the whole tile loop (graph does
    not grow) — used for timing with large in-NEFF repeat factors."""
    assert n_samples % 128 == 0
    n_mtiles = n_samples // 128

    nc = bass.Bass()

    x_d = nc.declare_dram_parameter("x", [n_samples, DIM], F32, isOutput=False)
    # x transposed per m-tile: [mt, feat_in_tile(p), ftile, b] fp8 (16*x)
    xtp_d = nc.declare_dram_parameter(
        "xtp", [n_mtiles, 128, 16, 128], FP8, isOutput=False)
    w_d = {}
    for name, pname in (("q1", "WQ1_w"), ("k1", "WK1_w"), ("q2", "WQ2_w"),
                        ("k2", "WK2_w"), ("v", "WV_w"), ("o", "WO_w")):
        w_d[name] = nc.declare_dram_parameter(pname, [SD, SD], FP8,
                                              isOutput=False)
    l1_d = nc.declare_dram_parameter("L1_w", [DIM, LH], FP8, isOutput=False)
    l2r_d = nc.declare_dram_parameter("L2r", [128, LH], F32, isOutput=False)
    out_d = nc.declare_dram_parameter("out", [n_samples, DIM], F32, isOutput=True)

    with TileContext(nc) as tc:
        with (
            tc.tile_pool(name="const", bufs=1) as const,
            tc.tile_pool(name="xnat", bufs=2) as xnat_p,
            tc.tile_pool(name="xt", bufs=2) as xt_p,
            tc.tile_pool(name="qk", bufs=2) as qk_p,
            tc.tile_pool(name="kdp", bufs=2) as kd_p,
            tc.tile_pool(name="vbuf", bufs=2) as v_p,
            tc.tile_pool(name="ptmp", bufs=2) as ptmp_p,
            tc.tile_pool(name="small", bufs=2) as small_p,
            tc.tile_pool(name="hbuf", bufs=2) as h_p,
            tc.tile_pool(name="dpool", bufs=32) as d_p,
            tc.tile_pool(name="at", bufs=2) as at_p,
            tc.tile_pool(name="obuf", bufs=2) as o_p,
            tc.tile_pool(name="psum_big", bufs=3, space="PSUM") as ps_big,
            tc.tile_pool(name="psum_lam", bufs=2, space="PSUM") as ps_lam_p,
        ):
            # ---------------- resident weights (already fp8, x64) -------------
            w_sb = {}
            for name in ("q1", "k1", "q2", "k2", "v", "o"):
                wt = const.tile([128, 8, SD], FP8, name=f"w_{name}")
                wr = w_d[name].rearrange("(ko p) n -> p ko n", p=128)
                # split so early-k matmuls can start before the full DMA lands
                nc.sync.dma_start(wt[:, :4, :], wr[:, :4, :])
                nc.sync.dma_start(wt[:, 4:, :], wr[:, 4:, :])
                w_sb[name] = wt
            l1_sb = const.tile([128, 16, LH], FP8, name="l1")
            nc.sync.dma_start(
                l1_sb[:], l1_d.rearrange("(ko p) n -> p ko n", p=128))
            l2_rep = const.tile([128, LH], F32, name="l2rep")
            nc.sync.dma_start(l2_rep[:], l2r_d[:])

            id_bf16 = const.tile([128, 128], BF16, name="id16")
            make_identity(nc, id_bf16[:])

            # ---------------- main loop over 128-sample tiles ----------------
            rep_cm = (tc.For_i(0, hw_repeats, 1) if hw_repeats > 1
                      else contextlib.nullcontext())
            with rep_cm:
              for mt_rep in range(n_mtiles * repeats):
                mt = mt_rep % n_mtiles
                r0 = mt * 128

                x_nat = xnat_p.tile([128, DIM], F32, tag="xnat", name="xnat")
                nc.sync.dma_start(x_nat[:], x_d[r0:r0 + 128, :])
                xt = xt_p.tile([128, 16, 128], FP8, tag="xt", name="xt")
                nc.sync.dma_start(xt[:], xtp_d[mt])

                # token difference of x^T: Kdiff_s = (T0 - T1) @ WKs directly
                xtd = ptmp_p.tile([128, 8, 128], FP8, tag="xtd", name="xtd")
                nc.vector.tensor_tensor(xtd[:], xt[:, 0:8, :], xt[:, 8:16, :],
                                        AluOp.subtract)
                kdiff = {}
                for kname in ("k1", "k2"):
                    psw = ps_big.tile([128, SD], F32, tag="pbig", name="pbig")
                    for i in range(4):
                        ksl = slice(2 * i, 2 * i + 2)
                        for n in range(2):
                            nsl = slice(n * 512, (n + 1) * 512)
                            nc.tensor.matmul(
                                psw[:, nsl], xtd[:, ksl, :],
                                w_sb[kname][:, ksl, nsl],
                                start=(i == 0), stop=(i == 3), perf_mode=DR)
                    kd = kd_p.tile([128, SD], BF16, tag=f"kd_{kname}",
                                   name=f"kd_{kname}")
                    kdiff[kname] = kd
                    nc.vector.tensor_copy(kd[:], psw[:])

                # Projections. Q/V per token in natural layout [b, f].
                # lam MLP hidden accumulates across both tokens, folded into
                # the q1 GEMM loops (reuses the x^T stationaries).
                ps_lam = ps_lam_p.tile([128, LH], F32, tag="plam", name="plam")
                proj = {}  # (w, tok) -> sbuf tile [128, 1024] bf16
                for tok in range(2):
                    for name in ("q1", "q2", "v"):
                        pool = v_p if name == "v" else qk_p
                        proj[(name, tok)] = pool.tile(
                            [128, SD], BF16, tag=f"qkv_{name}_{tok}",
                            name=f"qkv_{name}_{tok}")
                for tok in range(2):
                    psw = ps_big.tile([128, SD], F32, tag="pbig", name="pbig")
                    for i in range(4):
                        xsl = slice(tok * 8 + 2 * i, tok * 8 + 2 * i + 2)
                        wsl = slice(2 * i, 2 * i + 2)
                        lhsT = xt[:, xsl, :]
                        for n in range(2):
                            nsl = slice(n * 512, (n + 1) * 512)
                            nc.tensor.matmul(
                                psw[:, nsl], lhsT, w_sb["q1"][:, wsl, nsl],
                                start=(i == 0), stop=(i == 3), perf_mode=DR)
                        # lam MLP chunk rides on the same stationary
                        nc.tensor.matmul(
                            ps_lam[:], lhsT, l1_sb[:, xsl, :],
                            start=(tok == 0 and i == 0),
                            stop=(tok == 1 and i == 3), perf_mode=DR,
                            skip_group_check=True)
                    nc.vector.tensor_copy(proj[("q1", tok)][:], psw[:])
                for tok in range(2):
                    for name in ("q2", "v"):
                        psw = ps_big.tile([128, SD], F32, tag="pbig",
                                          name="pbig")
                        for i in range(4):
                            xsl = slice(tok * 8 + 2 * i, tok * 8 + 2 * i + 2)
                            wsl = slice(2 * i, 2 * i + 2)
                            for n in range(2):
                                nsl = slice(n * 512, (n + 1) * 512)
                                nc.tensor.matmul(
                                    psw[:, nsl], xt[:, xsl, :],
                                    w_sb[name][:, wsl, nsl],
                                    start=(i == 0), stop=(i == 3),
                                    perf_mode=DR)
                        if name == "v":
                            # evict with 1/QK_SCALE => natural-scale V bf16
                            nc.scalar.mul(proj[(name, tok)][:], psw[:],
                                          1.0 / QK_SCALE)
                        else:
                            nc.vector.tensor_copy(proj[(name, tok)][:], psw[:])

                # lambda = sigmoid(relu(H') . L2/1024)
                h_sb = h_p.tile([128, LH], F32, tag="h", name="h")
                nc.scalar.activation(h_sb[:], ps_lam[:], ActFn.Relu)
                nc.vector.tensor_tensor(h_sb[:], h_sb[:], l2_rep[:], AluOp.mult)
                logit = small_p.tile([128, 1], F32, tag="logit", name="logit")
                nc.vector.tensor_reduce(logit[:], h_sb[:],
                                        axis=mybir.AxisListType.X, op=AluOp.add)
                lam = small_p.tile([128, 1], F32, tag="lam", name="lam")
                nc.scalar.activation(lam[:], logit[:], ActFn.Sigmoid)
                u = small_p.tile([128, 1], F32, tag="u", name="u")
                nc.vector.tensor_scalar(u[:], lam[:], -1.0, 1.0,
                                        AluOp.mult, AluOp.add)

                # scores: r' = <Q'_q, Kd'> per head; a = sigmoid(SCALE/2^20 r')
                a = {}
                for si, sname in enumerate(("1", "2")):
                    for q in range(2):
                        p = ptmp_p.tile([128, SD], BF16, tag="p", name="p")
                        nc.vector.tensor_tensor(
                            p[:], proj[(f"q{sname}", q)][:],
                            kdiff[f"k{sname}"][:], AluOp.mult)
                        r = small_p.tile([128, H], F32, tag=f"r{si}{q}",
                                         name=f"r{si}{q}")
                        nc.vector.tensor_reduce(
                            r[:], p.rearrange("b (h d) -> b h d", h=H),
                            axis=mybir.AxisListType.X, op=AluOp.add)
                        aa = small_p.tile([128, H], F32, tag=f"a{si}{q}",
                                          name=f"a{si}{q}")
                        nc.scalar.activation(aa[:], r[:], ActFn.Sigmoid,
                                             scale=float(SCALE / QK_SCALE ** 2))
                        a[(si, q)] = aa

                # diff-attn weights -> diag matrices (bf16)
                dmats = {}
                for q in range(2):
                    t = small_p.tile([128, H], F32, tag=f"t{q}", name=f"t{q}")
                    nc.vector.tensor_scalar_mul(t[:], a[(1, q)][:], lam[:])
                    w0q = small_p.tile([128, H], F32, tag=f"w0{q}",
                                       name=f"w0{q}")
                    nc.vector.tensor_tensor(w0q[:], a[(0, q)][:], t[:],
                                            AluOp.subtract)
                    nc.vector.tensor_scalar_max(w0q[:], w0q[:], 0.0)
                    w1q = small_p.tile([128, H], F32, tag=f"w1{q}",
                                       name=f"w1{q}")
                    nc.vector.tensor_tensor(w1q[:], t[:], a[(0, q)][:],
                                            AluOp.subtract)
                    nc.vector.tensor_scalar(w1q[:], w1q[:], u[:], 0.0,
                                            AluOp.add, AluOp.max)
                    for h in range(H):
                        d0 = d_p.tile([128, 128], BF16, tag="dmat", name="dmat")
                        nc.vector.tensor_scalar_mul(
                            d0[:], id_bf16[:], w0q[:, h:h + 1])
                        d1 = d_p.tile([128, 128], BF16, tag="dmat", name="dmat")
                        nc.vector.tensor_scalar_mul(
                            d1[:], id_bf16[:], w1q[:, h:h + 1])
                        dmats[(q, h, 0)] = d0
                        dmats[(q, h, 1)] = d1

                # A_q^T via diag matmuls, 4 tiles packed per psum bank:
                #   A_q^T[ftile] = V_0[:,ft].T @ D0[q,h] + V_1[:,ft].T @ D1[q,h]
                # Evicted with x A_SCALE to fp8 => stationary for WO matmul.
                at = {}
                for q in range(2):
                    at[q] = at_p.tile([128, 8, 128], FP8, tag=f"at{q}",
                                      name=f"at{q}")
                    psq = ps_big.tile([128, SD], F32, tag="pbig", name="pbig")
                    for ft in range(8):
                        h = ft // 2
                        fsl = slice(ft * 128, (ft + 1) * 128)
                        for kv in range(2):
                            nc.tensor.matmul(
                                psq[:, fsl], proj[("v", kv)][:, fsl],
                                dmats[(q, h, kv)][:],
                                start=(kv == 0), stop=(kv == 1),
                                skip_group_check=(ft % 4 != 0))
                    nc.scalar.mul(at[q][:],
                                  psq[:].rearrange("b (f c) -> b f c", c=128),
                                  float(A_SCALE))

                # out_q = (A'_q @ WO')/512 + tokens_q   (fp8 DoubleRow)
                for q in range(2):
                    o_sb = o_p.tile([128, SD], F32, tag=f"o{q}", name=f"o{q}")
                    o_tmp = o_p.tile([128, SD], BF16, tag=f"ot{q}",
                                     name=f"ot{q}")
                    pso = ps_big.tile([128, SD], F32, tag="pbig", name="pbig")
                    for i in range(4):
                        for n in range(2):
                            nsl = slice(n * 512, (n + 1) * 512)
                            nc.tensor.matmul(pso[:, nsl],
                                             at[q][:, 2 * i:2 * i + 2, :],
                                             w_sb["o"][:, 2 * i:2 * i + 2, nsl],
                                             start=(i == 0), stop=(i == 3),
                                             perf_mode=DR)
                    nc.scalar.mul(o_tmp[:], pso[:], 1.0 / O_SCALE)
                    osl = slice(q * SD, (q + 1) * SD)
                    nc.vector.tensor_tensor(o_sb[:], o_tmp[:],
                                            x_nat[:, osl], AluOp.add)
                    nc.sync.dma_start(out_d[r0:r0 + 128, osl], o_sb[:])

    split_excess_waits(nc)
    return nc


_NC_CACHE = {}


def _get_nc(n_samples):
    if n_samples not in _NC_CACHE:
        _NC_CACHE[n_samples] = build_kernel(n_samples)
    return _NC_CACHE[n_samples]


def host_prep(inputs, n_samples=B_CORE):
    """Host-side shard + dtype/layout prep. Returns in_maps for 8 cores."""
    x = np.ascontiguousarray(np.asarray(inputs["x"], dtype=np.float32))
    assert x.shape[0] == N_CORES * n_samples and x.shape[1] == DIM
    f8 = ml_dtypes.float8_e4m3fn
    ws = {}
    for k in ("WQ1_w", "WK1_w", "WQ2_w", "WK2_w", "WV_w", "WO_w", "L1_w"):
        ws[k] = np.ascontiguousarray(
            (np.asarray(inputs[k], dtype=np.float32) * W_SCALE).astype(f8))
    l2rep = np.ascontiguousarray(
        np.broadcast_to(np.asarray(inputs["L2_w"], dtype=np.float32)
                        .reshape(1, LH) / QK_SCALE, (128, LH)))
    n_mtiles = n_samples // 128
    # scale+cast once (contiguous, fast), then one big strided transpose in
    # fp8: [core, mt, b, ft, p] -> [core, mt, p, ft, b]
    xb = (x * X_SCALE).astype(f8).reshape(N_CORES, n_mtiles, 128, 16, 128)
    xtp_all = np.ascontiguousarray(xb.transpose(0, 1, 4, 3, 2))
    in_maps = []
    for c in range(N_CORES):
        m = {"x": np.ascontiguousarray(x[c * n_samples:(c + 1) * n_samples]),
             "xtp": xtp_all[c], "L2r": l2rep}
        m.update(ws)
        in_maps.append(m)
    return in_maps


def kernel(**inputs):
    from concourse.bass_utils import run_bass_kernel_spmd

    nc = _get_nc(B_CORE)
    in_maps = host_prep(inputs, B_CORE)
    res = run_bass_kernel_spmd(nc, in_maps, core_ids=list(range(N_CORES)))
    return np.concatenate([res.results[c]["out"] for c in range(N_CORES)], axis=0)
